# revision 7
# baseline (speedup 1.0000x reference)
"""Mean-IoU kernel for Trainium2, SPMD over 8 NeuronCores.

Strategy (data-parallel over batch N=16, 2 images per core), v4:
  - NO host transpose: inputs stay class-planar (N, C, H*W), the native
    HBM layout. SWDGE DMA casts f32 -> bf16 on the way in, so on-chip
    tiles are x (128, 19, FT) bf16 with PIXELS innermost.
  - Per-pixel max over classes = chain of 18 DVE tensor_tensor(max) ops
    on contiguous (128, FT) bf16 slices. Both operands are step-1 bf16
    -> DVE 2x_1P mode (2 elem/cycle), 2x faster than a 1x tensor_reduce
    over a class-innermost layout.
  - One-hot zb = is_equal(x, m) with m broadcast along the MIDDLE class
    axis: inner step stays 1 on both operands -> 2x mode again.
    bf16 rounding can produce multi-hot rows (~0.7% of pixels); the
    resulting mean-IoU error is ~5e-5 (validated numerically), far
    under the 2e-2 tolerance.
  - TensorE bf16 matmuls Zp^T @ Zt accumulate a block confusion matrix:
    JB=4 pixel-columns x 19 classes -> 76x76 PSUM per image, columns
    ordered class-major (c*JB + j).
  - Host: sum j-diagonal of (19,4,19,4) blocks -> confusion M;
    pred = M.sum(1), targ = M.sum(0), inter = diag(M); IoU + means.
"""
import os
import sys

for _p in ('/opt/trn_rl_repo', '/root/.axon_site/_ro/trn_rl_repo'):
    if os.path.isdir(_p) and _p not in sys.path:
        sys.path.insert(0, _p)

import numpy as np

# problem constants (hardcoded per contest rules)
N_FULL = 16
C = 19
H = 512
W = 512
HW = H * W
EPS = 1e-06

N_CORES = 8
N_LOC = N_FULL // N_CORES      # 2 images per core
P = 128                        # SBUF partitions = pixel groups
Q = HW // P                    # 2048 pixels per partition
FT = 512                       # pixels per partition per chunk-tile
N_TILES = Q // FT              # 4 chunks per image
JB = 4                         # pixel-columns per confusion matmul
NCOLS = JB * C                 # 76

_CACHE = {}


def _build_nc():
    from concourse import bacc, tile, mybir

    nc = bacc.Bacc("TRN2", target_bir_lowering=False, debug=False,
                   num_devices=N_CORES)
    # native layout: (n, c, h*w)
    preds = nc.dram_tensor("preds", (N_LOC, C, HW), mybir.dt.float32,
                           kind="ExternalInput")
    targs = nc.dram_tensor("targets", (N_LOC, C, HW), mybir.dt.float32,
                           kind="ExternalInput")
    conf_out = nc.dram_tensor("conf", (N_LOC, NCOLS, NCOLS), mybir.dt.float32,
                              kind="ExternalOutput")

    # (n, c, p, q): partition p holds pixels [p*Q, (p+1)*Q) of each plane
    pvv = preds.ap().rearrange("n c (p q) -> n p c q", p=P)
    tvv = targs.ap().rearrange("n c (p q) -> n p c q", p=P)

    with tile.TileContext(nc) as tc:
        with (
            tc.tile_pool(name="sbuf", bufs=2) as pool,
            tc.tile_pool(name="psum", bufs=2, space="PSUM") as psum_pool,
        ):
            # tapered chunk schedule per image: small first chunk for a
            # fast pipeline ramp, small last chunk for a short tail after
            # the final DMA. Images interleave; buffers stay FT-sized and
            # small chunks use them partially.
            chunks = [(0, 256), (256, 512), (768, 512), (1280, 512),
                      (1792, 256)]
            order = [(n, s, l) for (s, l) in chunks for n in range(N_LOC)]
            confs = {}
            for n in range(N_LOC):
                cn = psum_pool.tile([NCOLS, NCOLS], mybir.dt.float32,
                                    tag=f"conf{n}")
                confs[n] = cn
            first_of = {n: min(i for i, o in enumerate(order) if o[0] == n)
                        for n in range(N_LOC)}
            last_of = {n: max(i for i, o in enumerate(order) if o[0] == n)
                       for n in range(N_LOC)}
            for i, (n, s, fl) in enumerate(order):
                conf = confs[n]
                zbs = {}
                for name, dview in (("p", pvv), ("t", tvv)):
                    x = pool.tile([P, C, FT], mybir.dt.bfloat16,
                                  tag=f"x{name}")
                    # SWDGE DMA with f32 -> bf16 cast
                    nc.gpsimd.dma_start(
                        x[:, :, 0:fl], dview[n, :, :, s:s + fl])
                    m = pool.tile([P, FT], mybir.dt.bfloat16,
                                  tag=f"m{name}")
                    nc.vector.tensor_tensor(
                        m[:, 0:fl], x[:, 0, 0:fl], x[:, 1, 0:fl],
                        op=mybir.AluOpType.max)
                    for c in range(2, C):
                        nc.vector.tensor_tensor(
                            m[:, 0:fl], m[:, 0:fl], x[:, c, 0:fl],
                            op=mybir.AluOpType.max)
                    # micro-tiled one-hot: (p, nb, c, j) so each JB-pixel
                    # block is a contiguous 76-column slab for the PE,
                    # while the DVE writes through a permuted view that
                    # still streams in input order with inner step 1.
                    zb = pool.tile([P, FT // JB, C, JB],
                                   mybir.dt.bfloat16, tag=f"zb{name}")
                    mb = m[:, None, 0:fl].broadcast_to((P, C, fl))
                    nc.vector.tensor_tensor(
                        zb[:, 0:fl // JB].rearrange("p nb c j -> p c nb j"),
                        x[:, :, 0:fl], mb, op=mybir.AluOpType.is_equal)
                    zbs[name] = zb
                for b in range(fl // JB):
                    first = (i == first_of[n] and b == 0)
                    last = (i == last_of[n] and b == fl // JB - 1)
                    nc.tensor.matmul(
                        conf[:],
                        zbs["p"][:, b].rearrange("p c j -> p (c j)"),
                        zbs["t"][:, b].rearrange("p c j -> p (c j)"),
                        start=first, stop=last)
            for n in range(N_LOC):
                sb = pool.tile([NCOLS, NCOLS], mybir.dt.float32,
                               tag="confsb")
                nc.scalar.copy(sb[:], confs[n][:])
                nc.sync.dma_start(conf_out.ap()[n], sb[:])

    nc.compile()
    return nc


def _get_nc():
    if "nc" not in _CACHE:
        _CACHE["nc"] = _build_nc()
    return _CACHE["nc"]


def run_on_hw(preds, targets, trace=False):
    """Run the SPMD kernel; returns (conf (16,NCOLS,NCOLS) np.f32, results)."""
    from concourse.bass_utils import run_bass_kernel_spmd

    nc = _get_nc()
    preds = np.ascontiguousarray(
        np.asarray(preds, dtype=np.float32).reshape(N_FULL, C, HW))
    targets = np.ascontiguousarray(
        np.asarray(targets, dtype=np.float32).reshape(N_FULL, C, HW))
    in_maps = [
        {"preds": preds[i * N_LOC:(i + 1) * N_LOC],
         "targets": targets[i * N_LOC:(i + 1) * N_LOC]}
        for i in range(N_CORES)
    ]
    res = run_bass_kernel_spmd(nc, in_maps, core_ids=list(range(N_CORES)),
                               trace=trace)
    conf = np.concatenate([res.results[i]["conf"] for i in range(N_CORES)],
                          axis=0)
    return conf, res


def postprocess(conf, class_weights):
    """conf: (16, NCOLS, NCOLS) block confusion -> scalar mean IoU.

    Column index = c*JB + j (class-major within a JB-pixel block);
    the per-class confusion sums the j-diagonal.
    """
    conf = conf.astype(np.float64).reshape(N_FULL, C, JB, C, JB)
    M = np.einsum('ncjdj->ncd', conf)
    inter = np.diagonal(M, axis1=1, axis2=2)          # (N, C)
    pred_cnt = M.sum(axis=2)                          # (N, C)
    targ_cnt = M.sum(axis=1)                          # (N, C)
    union = pred_cnt + targ_cnt - inter
    iou = (inter + EPS) / (union + EPS)
    weighted = iou * np.asarray(class_weights, dtype=np.float64)[None, :]
    return np.float32(weighted.mean())


def kernel(preds, targets, class_weights):
    conf, _ = run_on_hw(preds, targets, trace=False)
    return postprocess(conf, class_weights)


# revision 8
# speedup vs baseline: 1.0985x; 1.0985x over previous
"""Mean-IoU kernel for Trainium2, SPMD over 8 NeuronCores.

Strategy (data-parallel over batch N=16, 2 images per core), v4:
  - NO host transpose: inputs stay class-planar (N, C, H*W), the native
    HBM layout. SWDGE DMA casts f32 -> bf16 on the way in, so on-chip
    tiles are x (128, 19, FT) bf16 with PIXELS innermost.
  - Per-pixel max over classes = chain of 18 DVE tensor_tensor(max) ops
    on contiguous (128, FT) bf16 slices. Both operands are step-1 bf16
    -> DVE 2x_1P mode (2 elem/cycle), 2x faster than a 1x tensor_reduce
    over a class-innermost layout.
  - One-hot zb = is_equal(x, m) with m broadcast along the MIDDLE class
    axis: inner step stays 1 on both operands -> 2x mode again.
    bf16 rounding can produce multi-hot rows (~0.7% of pixels); the
    resulting mean-IoU error is ~5e-5 (validated numerically), far
    under the 2e-2 tolerance.
  - TensorE bf16 matmuls Zp^T @ Zt accumulate a block confusion matrix:
    JB=4 pixel-columns x 19 classes -> 76x76 PSUM per image, columns
    ordered class-major (c*JB + j).
  - Host: sum j-diagonal of (19,4,19,4) blocks -> confusion M;
    pred = M.sum(1), targ = M.sum(0), inter = diag(M); IoU + means.
"""
import os
import sys

for _p in ('/opt/trn_rl_repo', '/root/.axon_site/_ro/trn_rl_repo'):
    if os.path.isdir(_p) and _p not in sys.path:
        sys.path.insert(0, _p)

import numpy as np

# problem constants (hardcoded per contest rules)
N_FULL = 16
C = 19
H = 512
W = 512
HW = H * W
EPS = 1e-06

N_CORES = 8
N_LOC = N_FULL // N_CORES      # 2 images per core
P = 128                        # SBUF partitions = pixel groups
Q = HW // P                    # 2048 pixels per partition
FT = 512                       # pixels per partition per chunk-tile
N_TILES = Q // FT              # 4 chunks per image
JB = 4                         # pixel-columns per confusion matmul
NCOLS = JB * C                 # 76

_CACHE = {}


def _build_nc():
    from concourse import bacc, tile, mybir

    nc = bacc.Bacc("TRN2", target_bir_lowering=False, debug=False,
                   num_devices=N_CORES)
    # native layout: (n, c, h*w)
    preds = nc.dram_tensor("preds", (N_LOC, C, HW), mybir.dt.float32,
                           kind="ExternalInput")
    targs = nc.dram_tensor("targets", (N_LOC, C, HW), mybir.dt.float32,
                           kind="ExternalInput")
    conf_out = nc.dram_tensor("conf", (N_LOC, NCOLS, NCOLS), mybir.dt.float32,
                              kind="ExternalOutput")

    # (n, c, p, q): partition p holds pixels [p*Q, (p+1)*Q) of each plane
    pvv = preds.ap().rearrange("n c (p q) -> n p c q", p=P)
    tvv = targs.ap().rearrange("n c (p q) -> n p c q", p=P)

    with tile.TileContext(nc) as tc:
        with (
            tc.tile_pool(name="sbuf", bufs=2) as pool,
            tc.tile_pool(name="psum", bufs=2, space="PSUM") as psum_pool,
        ):
            for n in range(N_LOC):
                conf = psum_pool.tile([NCOLS, NCOLS], mybir.dt.float32,
                                      tag=f"conf{n}")
                for t in range(N_TILES):
                    zbs = {}
                    for name, dview in (("p", pvv), ("t", tvv)):
                        x = pool.tile([P, C, FT], mybir.dt.bfloat16,
                                      tag=f"x{name}")
                        # SWDGE DMA with f32 -> bf16 cast
                        nc.gpsimd.dma_start(
                            x[:], dview[n, :, :, t * FT:(t + 1) * FT])
                        m = pool.tile([P, FT], mybir.dt.bfloat16,
                                      tag=f"m{name}")
                        nc.vector.tensor_copy(m[:], x[:, 0, :])
                        for c in range(1, C):
                            nc.vector.tensor_tensor(
                                m[:], m[:], x[:, c, :],
                                op=mybir.AluOpType.max)
                        # micro-tiled one-hot: (p, nb, c, j) so each JB-pixel
                        # block is a contiguous 76-column slab for the PE,
                        # while the DVE writes through a permuted view that
                        # still streams in input order with inner step 1.
                        zb = pool.tile([P, FT // JB, C, JB],
                                       mybir.dt.bfloat16, tag=f"zb{name}")
                        mb = m[:, None, :].broadcast_to((P, C, FT))
                        nc.vector.tensor_tensor(
                            zb[:].rearrange("p nb c j -> p c nb j"),
                            x[:], mb, op=mybir.AluOpType.is_equal)
                        zbs[name] = zb
                    nmm = FT // JB                    # 128 uniform blocks
                    for b in range(nmm):
                        first = (t == 0 and b == 0)
                        last = (t == N_TILES - 1 and b == nmm - 1)
                        nc.tensor.matmul(
                            conf[:],
                            zbs["p"][:, b].rearrange("p c j -> p (c j)"),
                            zbs["t"][:, b].rearrange("p c j -> p (c j)"),
                            start=first, stop=last)
                sb = pool.tile([NCOLS, NCOLS], mybir.dt.float32,
                               tag="confsb")
                nc.scalar.copy(sb[:], conf[:])
                nc.sync.dma_start(conf_out.ap()[n], sb[:])

    nc.compile()
    return nc


def _get_nc():
    if "nc" not in _CACHE:
        _CACHE["nc"] = _build_nc()
    return _CACHE["nc"]


def run_on_hw(preds, targets, trace=False):
    """Run the SPMD kernel; returns (conf (16,NCOLS,NCOLS) np.f32, results)."""
    from concourse.bass_utils import run_bass_kernel_spmd

    nc = _get_nc()
    preds = np.ascontiguousarray(
        np.asarray(preds, dtype=np.float32).reshape(N_FULL, C, HW))
    targets = np.ascontiguousarray(
        np.asarray(targets, dtype=np.float32).reshape(N_FULL, C, HW))
    in_maps = [
        {"preds": preds[i * N_LOC:(i + 1) * N_LOC],
         "targets": targets[i * N_LOC:(i + 1) * N_LOC]}
        for i in range(N_CORES)
    ]
    res = run_bass_kernel_spmd(nc, in_maps, core_ids=list(range(N_CORES)),
                               trace=trace)
    conf = np.concatenate([res.results[i]["conf"] for i in range(N_CORES)],
                          axis=0)
    return conf, res


def postprocess(conf, class_weights):
    """conf: (16, NCOLS, NCOLS) block confusion -> scalar mean IoU.

    Column index = c*JB + j (class-major within a JB-pixel block);
    the per-class confusion sums the j-diagonal.
    """
    conf = conf.astype(np.float64).reshape(N_FULL, C, JB, C, JB)
    M = np.einsum('ncjdj->ncd', conf)
    inter = np.diagonal(M, axis1=1, axis2=2)          # (N, C)
    pred_cnt = M.sum(axis=2)                          # (N, C)
    targ_cnt = M.sum(axis=1)                          # (N, C)
    union = pred_cnt + targ_cnt - inter
    iou = (inter + EPS) / (union + EPS)
    weighted = iou * np.asarray(class_weights, dtype=np.float64)[None, :]
    return np.float32(weighted.mean())


def kernel(preds, targets, class_weights):
    conf, _ = run_on_hw(preds, targets, trace=False)
    return postprocess(conf, class_weights)
